# revision 63
# baseline (speedup 1.0000x reference)
"""MPNN layer (NNConv-style) Trainium2 Bass kernel, v3.

Strategy: shard by destination node. Core c owns nodes [c*6250, (c+1)*6250).
Host packs that core's edges (sorted by dst) into NG=50 sequential groups of
<=512 edges spanning <=128 consecutive nodes; nodes may split across group
boundaries (host sums the partial aggregates), so slot utilization is ~98%
with no spill. Host pre-gathers source features x and pre-transposes ef.

Per 128-edge tile on device (steady-state pipeline, group g = 4 tiles):
  h^T  = relu(W1^T @ ef^T + b1)      PE (two K=16 matmuls) + ACT relu
  we   = h^T' @ W2p                  PE, [128,1024] f32 PSUM (ring of 3)
  prod = we * x  -> bf16 SBUF, in four mirrored 256-col quarters, each
         [b: 2 i-vals o-major][pool: 3 i-vals][d: 3 i-vals] (sigma-permuted
         W2 columns make every engine's x-multipliers contiguous):
           b     ACT copy -> DVE tensor_tensor at 2x
           pool  ACT copy -> GPSIMD tensor_tensor
           d     DVE tensor_tensor direct from PSUM
  z   += onehot(rank)^T @ prod       PE: 16 folded matmuls accumulate the 4
         edge tiles AND the 4 quarters into one [128,256] PSUM tile (the
         mirrored quarters keep o aligned, so the fold is a valid segsum)
  agg  = reduce(z) on DVE (2 region reductions) -> DMA out (host adds both)
Host adds the b2 term (aggX @ b2r) and the output bias.

Engine balance per group (TimelineSim): PE 3.63us, ACT 3.58us, Pool 3.44us,
DVE 3.39us; makespan 205.6us vs 318.5us for the v2 baseline (PE ~89% busy;
residual idle is ~4.5us pipeline fill at the DMA-chain latency floor, ~5us
drain through the final zred+DMA chain, and ~240ns/cycle of We->mult->Z
chain latency that ordering cannot remove).
"""

import sys

for _p in ("/opt/trn_rl_repo",):
    if _p not in sys.path:
        sys.path.insert(0, _p)

import numpy as np

N_NODES = 50000
N_EDGES = 200000
HID = 32
ED = 16
EH = 128
NCORES = 8
NPC = N_NODES // NCORES  # 6250 nodes per core
EDGE_CAP = 512  # edges per group
NODE_SPAN = 128  # max node span per group (rank width)
NG = 50  # groups per core (seed-0 data needs exactly 50 on every core)
NT = NG * 4  # 200 tiles per core
CHG = 2  # groups per DMA chunk
NCH = NG // CHG  # 10 chunks

# prod column layout (after the sigma i-permutation in W2p): four mirrored
# 256-col quarters, each [b: 3 i-values o-major][pool: 2 i-values i-major]
# [d: 3 i-values i-major]. ACT copies b+pool (cols 0:160 of each quarter),
# DVE multiplies b at 2x from the copy and d directly from PSUM, GPSIMD
# multiplies pool. The fold over quarters preserves o, so one [128, 256]
# PSUM tile accumulates the segment sums.
BQ = 2   # b i-values per quarter (o-major, DVE 2x)
PQ = 3   # pool i-values per quarter
DQ = 3   # d i-values per quarter (DVE direct from PSUM)
BC = BQ * 32   # 96
PC = PQ * 32   # 64
CPQ = BC + PC  # 160 copied cols per quarter

_prog_cache = {}


def _build_program():
    import concourse.bacc as bacc
    import concourse.mybir as mybir
    from concourse.tile import TileContext

    f32 = mybir.dt.float32
    bf = mybir.dt.bfloat16
    AF = mybir.ActivationFunctionType
    ALU = mybir.AluOpType
    AX = mybir.AxisListType

    nc = bacc.Bacc(
        "TRN2", target_bir_lowering=False, debug=False, num_devices=NCORES
    )
    S_d = nc.dram_tensor("Sall", [128, NT * 128], bf, kind="ExternalInput")
    x_d = nc.dram_tensor("xsl", [128, NT * 32], bf, kind="ExternalInput")
    efT_d = nc.dram_tensor("efT", [16, NG * 512], bf, kind="ExternalInput")
    W1_d = nc.dram_tensor("W1b", [128, 4 * EH], bf, kind="ExternalInput")
    b1_d = nc.dram_tensor("b1c", [EH, 2], f32, kind="ExternalInput")
    W2_d = nc.dram_tensor("W2p", [EH, HID * HID], bf, kind="ExternalInput")
    agg_d = nc.dram_tensor(
        "aggout", [NG * 128, 2 * HID], f32, kind="ExternalOutput"
    )

    with TileContext(nc) as tc:
        with (
            tc.tile_pool(name="const", bufs=1) as cp,
            tc.tile_pool(name="sch", bufs=3) as sp,
            tc.tile_pool(name="ech", bufs=3) as ep,
            tc.tile_pool(name="xch", bufs=3) as xp,
            tc.tile_pool(name="hsb", bufs=2) as hp,
            tc.tile_pool(name="wsb", bufs=12) as wp,
            tc.tile_pool(name="prod", bufs=16) as pp,
            tc.tile_pool(name="aggs", bufs=3) as ap_,
            tc.tile_pool(name="ps_h", bufs=1, space="PSUM") as ps_h,
            tc.tile_pool(name="ps_we", bufs=3, space="PSUM") as ps_we,
            tc.tile_pool(name="ps_z", bufs=1, space="PSUM") as ps_z,
        ):
            W1_sb = cp.tile([128, 4 * EH], bf)
            b1_sb = cp.tile([EH, 2], f32)
            W2_sb = cp.tile([EH, HID * HID], bf)

            s_ch = {}
            e_ch = {}
            x_ch = {}

            def load_chunk(chi):
                # ef first: the h matmuls are the first consumers, and the
                # prologue critical path runs through this DMA
                e_ch[chi] = ep.tile([16, CHG * 512], bf, tag="ef", name=f"ech{chi}")
                nc.sync.dma_start(
                    out=e_ch[chi][:],
                    in_=efT_d[:, chi * CHG * 512 : (chi + 1) * CHG * 512],
                )
                x_ch[chi] = xp.tile([128, CHG * 128], bf, tag="x", name=f"xch{chi}")
                nc.sync.dma_start(
                    out=x_ch[chi][:],
                    in_=x_d[:, chi * CHG * 128 : (chi + 1) * CHG * 128],
                )
                s_ch[chi] = sp.tile([128, CHG * 512], bf, tag="S", name=f"sch{chi}")
                nc.sync.dma_start(
                    out=s_ch[chi][:],
                    in_=S_d[:, chi * CHG * 512 : (chi + 1) * CHG * 512],
                )

            # prologue order: group-0 ef before the big constants, S last
            # Spread the prologue DMAs across engine queues so their DGE
            # chains start concurrently (SP would serialize them 565ns apart)
            nc.scalar.dma_start(
                out=e_ch.setdefault(
                    0, ep.tile([16, CHG * 512], bf, tag="ef", name="ech0")
                )[:],
                in_=efT_d[:, 0 : CHG * 512],
            )
            nc.sync.dma_start(out=W1_sb[:], in_=W1_d[:])
            nc.sync.dma_start(out=b1_sb[:], in_=b1_d[:])
            nc.sync.dma_start(out=W2_sb[:], in_=W2_d[:])
            nc.sync.dma_start(
                out=x_ch.setdefault(
                    0, xp.tile([128, CHG * 128], bf, tag="x", name="xch0")
                )[:],
                in_=x_d[:, 0 : CHG * 128],
            )
            nc.sync.dma_start(
                out=s_ch.setdefault(
                    0, sp.tile([128, CHG * 512], bf, tag="S", name="sch0")
                )[:],
                in_=S_d[:, 0 : CHG * 512],
            )

            h_sbs = {}
            h_pss = {}

            def emit_h_mm(g):
                """h matmul for group g: one K=16 matmul over all 4 tiles."""
                chi = g // CHG
                gl = g % CHG
                h_pss[g] = ps_h.tile([EH, 512], f32, tag="h", name=f"hps{g}")
                for hh in range(2):
                    nc.tensor.matmul(
                        out=h_pss[g][:, hh * 256 : (hh + 1) * 256],
                        lhsT=W1_sb[0:16, 0:EH],
                        rhs=e_ch[chi][
                            0:16, gl * 512 + hh * 256 : gl * 512 + (hh + 1) * 256
                        ],
                        start=True, stop=True,
                    )

            def emit_relu(g):
                # first in the ACT queue each cycle, ahead of the we-copies;
                # split in two halves so the next group's first h matmuls
                # (WAR on h_ps) can start as soon as the low half is read
                h_ps = h_pss.pop(g)
                h_sbs[g] = hp.tile([EH, 512], bf, tag="h", name=f"h{g}")
                nc.scalar.activation(
                    out=h_sbs[g][:, 0:256], in_=h_ps[:, 0:256], func=AF.Relu,
                    bias=b1_sb[:, 0:1], scale=1.0,
                )
                nc.scalar.activation(
                    out=h_sbs[g][:, 256:512], in_=h_ps[:, 256:512],
                    func=AF.Relu, bias=b1_sb[:, 0:1], scale=1.0,
                )

            prods = {}

            def emit_we(g, t):
                """We matmuls for tile t of group g."""
                we = ps_we.tile([128, 1024], f32, tag="we", name=f"we{4 * g + t}")
                nc.tensor.matmul(
                    out=we[:, 0:512],
                    lhsT=h_sbs[g][:, t * 128 : (t + 1) * 128],
                    rhs=W2_sb[:, 0:512],
                    start=True, stop=True,
                )
                nc.tensor.matmul(
                    out=we[:, 512:1024],
                    lhsT=h_sbs[g][:, t * 128 : (t + 1) * 128],
                    rhs=W2_sb[:, 512:1024],
                    start=True, stop=True,
                )
                return we

            def emit_mult(g, t, we):
                """ACT copies b+pool slabs; DVE (b at 2x, d direct) and
                GPSIMD (pool) multiply by x per the sigma packing."""
                T = 4 * g + t
                chi = g // CHG
                tloc = T - chi * CHG * 4
                x_t = x_ch[chi][:, tloc * 32 : (tloc + 1) * 32]
                pr = pp.tile([128, 1024], bf, tag="prod", name=f"pr{T}")
                prods[T] = pr
                pr4 = pr[:].rearrange("p (q c) -> p q c", q=4)
                we4 = we[:].rearrange("p (q c) -> p q c", q=4)
                # d: direct from PSUM, i-major
                nc.vector.tensor_tensor(
                    out=pr4[:, :, CPQ:256].rearrange("p q (i o) -> p q i o", o=32),
                    in0=we4[:, :, CPQ:256].rearrange("p q (i o) -> p q i o", o=32),
                    in1=x_t[:, 4 * (BQ + PQ) : 32].rearrange("p (q i) -> p q i", q=4)[
                        :, :, :, None
                    ].to_broadcast([128, 4, DQ, 32]),
                    op=ALU.mult,
                )
                # b+pool evacuation
                wsb = wp.tile([128, 4 * CPQ], bf, tag="we", name=f"wsb{T}")
                wsb4 = wsb[:].rearrange("p (q c) -> p q c", q=4)
                nc.scalar.copy(out=wsb4, in_=we4[:, :, 0:CPQ])
                # b: DVE 2x from the copy, o-major
                nc.vector.tensor_tensor(
                    out=pr4[:, :, 0:BC].rearrange("p q (o i) -> p q o i", i=BQ),
                    in0=wsb4[:, :, 0:BC].rearrange("p q (o i) -> p q o i", i=BQ),
                    in1=x_t[:, 0 : 4 * BQ].rearrange("p (q i) -> p q i", q=4)[
                        :, :, None, :
                    ].to_broadcast([128, 4, 32, BQ]),
                    op=ALU.mult,
                )
                # pool: GPSIMD from the copy, i-major
                nc.gpsimd.tensor_tensor(
                    out=pr4[:, :, BC:CPQ].rearrange("p q (i o) -> p q i o", o=32),
                    in0=wsb4[:, :, BC:CPQ].rearrange("p q (i o) -> p q i o", o=32),
                    in1=x_t[:, 4 * BQ : 4 * (BQ + PQ)].rearrange("p (q i) -> p q i", q=4)[
                        :, :, :, None
                    ].to_broadcast([128, 4, PQ, 32]),
                    op=ALU.mult,
                )

            z_tiles = {}

            def _s_slice(g, t):
                chi = g // CHG
                tloc = 4 * g + t - chi * CHG * 4
                return s_ch[chi][:, tloc * 128 : (tloc + 1) * 128]

            def emit_z(g, ts):
                """Folded Z accumulation for tiles ts of group g into the
                group's [128, 256] PSUM tile."""
                if 0 in ts:
                    z_tiles[g] = ps_z.tile(
                        [128, 256], f32, tag="z", name=f"z{g}"
                    )
                z = z_tiles[g]
                for t in ts:
                    pr = prods[4 * g + t] if t < 3 else prods.pop(4 * g + t)
                    if t == 3:
                        for tt in range(3):
                            prods.pop(4 * g + tt, None)
                    S_sl = _s_slice(g, t)
                    for ci in range(4):
                        nc.tensor.matmul(
                            out=z[:], lhsT=S_sl,
                            rhs=pr[:, ci * 256 : (ci + 1) * 256],
                            start=(t == 0 and ci == 0),
                            stop=(t == 3 and ci == 3),
                        )

            outbufs = {}

            def emit_zred(g):
                """Reduce the folded z quarter-structure -> two partials
                (i-major region, o-major region); the host adds them."""
                z = z_tiles.pop(g)
                agg_sb = ap_.tile([128, 2 * HID], f32, tag="agg", name=f"agg{g}")
                nc.vector.tensor_reduce(
                    out=agg_sb[:, 0:HID],
                    in_=z[:, BC:256].rearrange("p (i o) -> p o i", i=PQ + DQ),
                    axis=AX.X, op=ALU.add,
                )
                nc.vector.tensor_reduce(
                    out=agg_sb[:, HID : 2 * HID],
                    in_=z[:, 0:BC].rearrange("p (o i) -> p o i", i=BQ),
                    axis=AX.X, op=ALU.add,
                )
                outbufs[g] = agg_sb

            def emit_out_dma(g):
                agg_sb = outbufs.pop(g)
                nc.sync.dma_start(
                    out=agg_d[g * 128 : (g + 1) * 128, :], in_=agg_sb[:]
                )

            emit_h_mm(0)
            emit_relu(0)
            for j in range(NG + 2):
                if j < NG and j % CHG == 0 and j // CHG + 1 < NCH:
                    load_chunk(j // CHG + 1)
                if j >= 2:
                    emit_zred(j - 2)
                    emit_out_dma(j - 2)
                if j < NG:
                    if j + 1 < NG:
                        emit_h_mm(j + 1)
                        emit_relu(j + 1)
                    emit_mult(j, 0, emit_we(j, 0))
                    emit_mult(j, 1, emit_we(j, 1))
                    if j >= 1:
                        emit_z(j - 1, (0, 1))
                    emit_mult(j, 2, emit_we(j, 2))
                    if j >= 1:
                        emit_z(j - 1, (2, 3))
                    emit_mult(j, 3, emit_we(j, 3))
                elif j == NG:
                    emit_z(j - 1, (0, 1))
                    emit_z(j - 1, (2, 3))
    nc.compile()
    return nc


def _layout_core(edge_src, edge_dst, ef_bf, nf_bf, c):
    """Sequential group packing for core c: groups of <=512 edges spanning
    <=128 consecutive dst nodes; nodes may split across groups."""
    sel = np.nonzero((edge_dst // NPC) == c)[0]
    dl_all = edge_dst[sel].astype(np.int64) - c * NPC
    order = np.argsort(dl_all, kind="stable")
    se = sel[order]
    dl = dl_all[order]
    n = len(se)

    bounds = []  # (e0, e1, n0)
    i = 0
    while i < n:
        n0 = int(dl[i])
        j = min(i + EDGE_CAP, n, int(np.searchsorted(dl, n0 + NODE_SPAN)))
        bounds.append((i, j, n0))
        i = j
    G = len(bounds)
    spill = []
    if G > NG:  # capacity overflow: host-compute the tail
        cut = bounds[NG][0]
        spill = list(se[cut:])
        se, dl = se[:cut], dl[:cut]
        bounds = bounds[:NG]
        G = NG

    bfl = ef_bf.dtype
    S = np.zeros((128, NT, 128), dtype=bfl)
    xsl = np.zeros((128, NT, HID), dtype=bfl)
    efsl = np.zeros((NT * 128, ED), dtype=bfl)
    n0s = np.zeros(G, dtype=np.int64)
    spans = np.zeros(G, dtype=np.int64)
    for g, (e0, e1, n0) in enumerate(bounds):
        cnt = e1 - e0
        sl = np.arange(cnt)
        row = sl & 127
        tile = 4 * g + (sl >> 7)
        rank = dl[e0:e1] - n0
        S[row, tile, rank] = 1
        xsl[row, tile] = nf_bf[edge_src[se[e0:e1]]]
        efsl[g * 512 + sl] = ef_bf[se[e0:e1]]
        n0s[g] = n0
        spans[g] = int(dl[e1 - 1]) - n0 + 1

    eft = np.ascontiguousarray(
        efsl.reshape(NG * 512, ED).T.reshape(ED, NG * 512)
    )

    dev = {
        "Sall": np.ascontiguousarray(S.reshape(128, NT * 128)),
        "xsl": np.ascontiguousarray(xsl.reshape(128, NT * HID)),
        "efT": eft,
    }
    return dev, (n0s, spans, G), spill


def _make_in_maps(nf, ef, edge_src, edge_dst, W1, b1, W2, b2, bias):
    import ml_dtypes

    bfl = ml_dtypes.bfloat16
    nf_bf = nf.astype(bfl)
    ef_bf = ef.astype(bfl)
    # W2p quarter layout (col = 256q + qc): qc in [0:BC) b-region o-major
    # (qc = BQ*o + ib, orig i = BQ*q + ib); qc in [BC:CPQ) pool i-major
    # (orig i = 4*BQ + PQ*q + ip); qc in [CPQ:256) d i-major (orig
    # i = 4*(BQ+PQ) + DQ*q + id).
    W2r3 = W2.reshape(EH, HID, HID)
    W2p = np.zeros((EH, HID * HID), dtype=np.float32)
    o_idx = np.arange(HID)
    for q in range(4):
        base = 256 * q
        for ib in range(BQ):
            W2p[:, base + BQ * o_idx + ib] = W2r3[:, BQ * q + ib, :]
        for ip_ in range(PQ):
            W2p[:, base + BC + 32 * ip_ : base + BC + 32 * ip_ + 32] = W2r3[
                :, 4 * BQ + PQ * q + ip_, :
            ]
        for id_ in range(DQ):
            W2p[:, base + CPQ + 32 * id_ : base + CPQ + 32 * id_ + 32] = W2r3[
                :, 4 * (BQ + PQ) + DQ * q + id_, :
            ]
    W2p = W2p.astype(bfl)
    W1r = np.zeros((128, 4 * EH), dtype=bfl)
    W1r[0:ED, 0:EH] = W1.astype(bfl)
    common = {
        "W1b": W1r,
        "b1c": np.ascontiguousarray(np.tile(b1.reshape(EH, 1), (1, 2))),
        "W2p": np.ascontiguousarray(W2p),
    }
    in_maps, remaps, spill = [], [], []
    for c in range(NCORES):
        dev, remap, sp = _layout_core(edge_src, edge_dst, ef_bf, nf_bf, c)
        in_maps.append({**common, **dev})
        remaps.append(remap)
        spill.extend(sp)
    return in_maps, remaps, spill


def kernel(nf, ef, edge_src, edge_dst, W1, b1, W2, b2, bias):
    from concourse.bass_utils import run_bass_kernel_spmd

    nf = np.asarray(nf, dtype=np.float32)
    ef = np.asarray(ef, dtype=np.float32)
    edge_src = np.asarray(edge_src, dtype=np.int32)
    edge_dst = np.asarray(edge_dst, dtype=np.int32)
    W1 = np.asarray(W1, dtype=np.float32)
    b1 = np.asarray(b1, dtype=np.float32)
    W2 = np.asarray(W2, dtype=np.float32)
    b2 = np.asarray(b2, dtype=np.float32)
    bias = np.asarray(bias, dtype=np.float32)

    if "prog" not in _prog_cache:
        _prog_cache["prog"] = _build_program()
    nc = _prog_cache["prog"]

    in_maps, remaps, spill = _make_in_maps(
        nf, ef, edge_src, edge_dst, W1, b1, W2, b2, bias
    )
    res = run_bass_kernel_spmd(nc, in_maps, core_ids=list(range(NCORES)))

    b2r = b2.reshape(HID, HID)
    # b2 term: aggX[d] = sum_{e->d} nf[src[e]] computed host-side (exact)
    order = np.argsort(edge_dst, kind="stable")
    sdst = edge_dst[order]
    seg_starts = np.nonzero(
        np.concatenate(([True], sdst[1:] != sdst[:-1]))
    )[0]
    seg_nodes = sdst[seg_starts]
    aggX = np.zeros((N_NODES, HID), dtype=np.float32)
    aggX[seg_nodes] = np.add.reduceat(nf[edge_src[order]], seg_starts, axis=0)
    out = aggX @ b2r + bias[None, :]
    for c in range(NCORES):
        n0s, spans, G = remaps[c]
        agg = np.asarray(res.results[c]["aggout"], dtype=np.float32)
        for g in range(G):
            sp_ = int(spans[g])
            rows = agg[g * 128 : g * 128 + sp_]
            out[c * NPC + n0s[g] : c * NPC + n0s[g] + sp_] += (
                rows[:, :HID] + rows[:, HID:]
            )

    if spill:  # capacity spill: finish the stragglers on host
        e = np.asarray(spill, dtype=np.int64)
        h = np.maximum(ef[e] @ W1 + b1, 0.0)
        We = (h @ W2).reshape(-1, HID, HID)
        msg = np.einsum("ei,eio->eo", nf[edge_src[e]], We)
        np.add.at(out, edge_dst[e], msg)

    return np.ascontiguousarray(out, dtype=np.float32)
